# revision 52
# baseline (speedup 1.0000x reference)
"""Batched tridiagonal (Thomas) solve on 8 TRN2 NeuronCores.

System per row (alpha in [0, 0.3)):
    sub a_i = alpha_{i-1}^2, diag b_i = 1 + alpha_i^3, super c_i = alpha_{i+1}^2 + 2 alpha_{i+1}
Forward elimination denominators denom_i = b_i - g_i/denom_{i-1} (g_i = a_i c_{i-1})
are computed via the linear scan d_i = g_i d_{i-1} + (b_i - 2 g_i), using
1/x ~= 2 - x near 1 (valid: diagonal dominance keeps denom in [0.93, 1.03];
measured end-to-end rel err ~1e-5).  cp/dp/u then come from first-order
recurrences executed with the hardware tensor_tensor_scan instruction.

Sharding: pure data parallel over batch rows (256 rows/core).  Within a core,
rows are split into 128-partition blocks and columns into strips with
contraction halos (forward influence decays ~0.096/step, backward ~0.74/step),
making every (block, strip) job fully independent.
"""

import sys

sys.path.insert(0, "/opt/trn_rl_repo")

import numpy as np

from concourse import bacc, mybir, tile
from concourse import bass_utils
from concourse.ap import AP as bass_AP

F32 = mybir.dt.float32
BF16 = mybir.dt.bfloat16
OP = mybir.AluOpType

B, N = 2048, 8192
NCORES = 8
RPC = B // NCORES          # rows per core
PB = 128                   # partition block (rows per job)
STRIP = 1024               # output columns per job
HALO_L = 16                # forward-scan warmup (contraction <= 0.0964/step)
HALO_R = 48                # backward-scan warmup (contraction <= 0.739/step)


def _act_reciprocal(nc, out, in_, scale=1.0, bias=0.0):
    """ACT Reciprocal: out = 1/(scale*in). Emitted directly (the bass wrapper
    refuses Reciprocal for generic accuracy reasons; on our inputs, |d| in
    [0.93, 1.03], HW-measured max rel err is 1.2e-5)."""
    se = nc.scalar
    return se.add_instruction(
        mybir.InstActivation(
            name=nc.get_next_instruction_name(),
            func=mybir.ActivationFunctionType.Reciprocal,
            ins=[
                se.lower_ap(in_),
                mybir.ImmediateValue(dtype=mybir.dt.float32, value=bias),
                mybir.ImmediateValue(dtype=mybir.dt.float32, value=scale),
                mybir.ImmediateValue(dtype=mybir.dt.float32, value=0.0),
            ],
            outs=[se.lower_ap(out)],
        )
    )


def build_core_program(nc, rows=RPC, n=N, strip=STRIP, halo_l=HALO_L, halo_r=HALO_R,
                       bufs=6, fr_mode="pool", b_act=True, rnh_act=True,
                       mid_lag=1, back_lag=2, c_alt=0):
    alpha_d = nc.dram_tensor("alpha", [rows, n], F32, kind="ExternalInput").ap()
    fbig_d = nc.dram_tensor("fbig", [PB, n], F32, kind="ExternalInput").ap()
    out_d = nc.dram_tensor("out", [rows, n], F32, kind="ExternalOutput").ap()

    n_blocks = (rows + PB - 1) // PB
    n_strips = (n + strip - 1) // strip
    wmax = halo_l + strip + halo_r

    with tile.TileContext(nc) as tc:
        with tc.tile_pool(name="cpool", bufs=1) as cpool:
            ones = None
            if c_alt:
                ones = cpool.tile([PB, wmax + 2], F32, tag="ones", name="t_ones")
                nc.gpsimd.memset(ones[:], 1.0)
            jobs = []
            for blk in range(n_blocks):
                for si in range(n_strips):
                    jobs.append((blk * PB, si * strip))

            def front(pool, r0, s, jidx=0):
                """DMA + coefficient prep, through g and w."""
                # uniform domain width: edge strips extend their halo inward,
                # so pad columns sit at fixed offsets and slots stay zeroed
                # after their first use.
                w = min(n, wmax)
                dom_lo = max(0, min(s - halo_l, n - w))
                dom_hi = dom_lo + w
                j = {
                    "w": w, "oo": s - dom_lo, "r0": r0, "s": s,
                    "dom_lo": dom_lo, "dom_hi": dom_hi,
                    # padded buffers: col 0 / col w+1 are zero pads for the
                    # shifted reads g_k = A2[k-1]*C[k], ncp_k = -C[k+1]*r_k.
                    "at": pool.tile([PB, wmax + 2], F32, tag="alpha", name="t_alpha"),
                    "a2h": pool.tile([PB, wmax + 2], BF16, tag="a2h", name="t_a2h"),
                    "ch": pool.tile([PB, wmax + 2], BF16, tag="ch", name="t_ch"),
                    "gt": pool.tile([PB, wmax], BF16, tag="g", name="t_g"),
                    "a3t": pool.tile([PB, wmax], F32, tag="a3", name="t_a3"),
                    "bt": pool.tile([PB, wmax + 2], F32, tag="b", name="t_b"),
                    "wt": pool.tile([PB, wmax], F32, tag="w", name="t_w"),
                    "rnh": pool.tile([PB, wmax], BF16, tag="rn", name="t_rn"),
                    "dp": pool.tile([PB, wmax], F32, tag="dp", name="t_dp"),
                    "fbj": pool.tile([PB, wmax], F32, tag="fbj", name="t_fbj"),
                }
                at, a2h, ch = j["at"], j["a2h"], j["ch"]
                nc.gpsimd.memset(at[:, 0:1], 0.0)
                nc.gpsimd.memset(at[:, w + 1 : w + 2], 0.0)
                nc.sync.dma_start(
                    out=at[:, 1 : w + 1], in_=alpha_d[r0 : r0 + PB, dom_lo:dom_hi]
                )
                nc.sync.dma_start(
                    out=j["fbj"][:, 0:w], in_=fbig_d[:, dom_lo:dom_hi]
                )
                # A2 (bf16), S = (alpha+1)^2  (ACT)
                nc.scalar.square(a2h[:, 0 : w + 2], at[:, 0 : w + 2])
                st = j["bt"]  # S staged in b's buffer
                nc.scalar.activation(
                    st[:, 0 : w + 2], at[:, 0 : w + 2],
                    mybir.ActivationFunctionType.Square, bias=1.0, scale=1.0,
                )
                # C = S - 1 = 2 alpha + alpha^2  (bf16; alternate jobs on Pool)
                if c_alt and jidx % 2 == 0:
                    nc.gpsimd.tensor_tensor(
                        out=ch[:, 0 : w + 2], in0=st[:, 0 : w + 2],
                        in1=ones[:, 0 : w + 2], op=OP.subtract,
                    )
                else:
                    nc.vector.tensor_scalar(
                        out=ch[:, 0 : w + 2], in0=st[:, 0 : w + 2], scalar1=-1.0,
                        scalar2=None, op0=OP.add,
                    )
                # A3 = alpha * A2 (Pool, mixed f32 x bf16), b = A3 + 1
                nc.gpsimd.tensor_tensor(
                    out=j["a3t"][:, 0:w], in0=at[:, 1 : w + 1],
                    in1=a2h[:, 1 : w + 1], op=OP.mult,
                )
                # bm2 = b - 2 = A3 - 1  (z-scan right-hand side)
                if b_act:
                    # Copy takes an immediate bias (same path as Reciprocal,
                    # HW-verified); Identity would need a registered const AP.
                    nc.scalar.activation(
                        j["bt"][:, 0:w], j["a3t"][:, 0:w],
                        mybir.ActivationFunctionType.Copy, bias=-1.0, scale=1.0,
                    )
                else:
                    nc.vector.tensor_scalar(
                        out=j["bt"][:, 0:w], in0=j["a3t"][:, 0:w], scalar1=-1.0,
                        scalar2=None, op0=OP.add,
                    )
                # g_k = A2[k-1] * C[k]  (bf16 2x)
                nc.vector.tensor_tensor(
                    out=j["gt"][:, 0:w], in0=a2h[:, 0:w], in1=ch[:, 1 : w + 1],
                    op=OP.mult,
                )
                return j

            def mid(j):
                """z-scan (z = d - 2), then rn = 1/(-z-2) = -1/d on ACT."""
                w = j["w"]
                zt = j["a3t"]  # A3 dead after bm2
                nc.vector.tensor_tensor_scan(
                    out=zt[:, 0:w], data0=j["gt"][:, 0:w], data1=j["bt"][:, 0:w],
                    initial=0.0, op0=OP.mult, op1=OP.add,
                )
                rn = j["bt"]  # bm2 dead after z-scan; rn = -1/d (f32)
                _act_reciprocal(nc, rn[:, 0:w], zt[:, 0:w], scale=-1.0, bias=-2.0)
                if rnh_act:
                    nc.scalar.mul(j["rnh"][:, 0:w], rn[:, 0:w], 1.0)
                else:
                    nc.vector.tensor_scalar(
                        out=j["rnh"][:, 0:w], in0=rn[:, 0:w], scalar1=1.0,
                        scalar2=None, op0=OP.mult,
                    )

            def back(j):
                """ar', fr', dp-scan, ncp, u-scan, output DMA."""
                w, r0, s = j["w"], j["r0"], j["s"]
                at, a2h, ch, gt, rn = j["at"], j["a2h"], j["ch"], j["gt"], j["bt"]
                # ar'_k = A2[k-1] * rn_k  (bf16 2x, into gt; g dead)
                nc.vector.tensor_tensor(
                    out=gt[:, 0:w], in0=a2h[:, 0:w], in1=j["rnh"][:, 0:w],
                    op=OP.mult,
                )
                # fr'_k = f_k * rn_k = -f_k r_k  (into wt; w dead)
                fr = j["wt"]
                eng = nc.gpsimd if fr_mode == "pool" else nc.vector
                eng.tensor_tensor(
                    out=fr[:, 0:w], in0=j["fbj"][:, 0:w],
                    in1=rn[:, 0:w], op=OP.mult,
                )
                # dp-scan: dp_k = ar'_k * dp_{k-1} - fr'_k  (dp positive)
                nc.vector.tensor_tensor_scan(
                    out=j["dp"][:, 0:w], data0=gt[:, 0:w], data1=fr[:, 0:w],
                    initial=0.0, op0=OP.mult, op1=OP.subtract,
                )
                # ncp_k = C[k+1] * rn_k  (bf16 2x, into a2h; dead after ar')
                ncp = a2h
                nc.vector.tensor_tensor(
                    out=ncp[:, 0:w], in0=ch[:, 2 : w + 2], in1=j["rnh"][:, 0:w],
                    op=OP.mult,
                )
                # u-scan (backward): u_k = ncp_k * u_{k+1} + dp_k  (into wt)
                ut = j["wt"]
                nc.vector.tensor_tensor_scan(
                    out=ut[:, 0:w][:, ::-1],
                    data0=ncp[:, 0:w][:, ::-1],
                    data1=j["dp"][:, 0:w][:, ::-1],
                    initial=0.0, op0=OP.mult, op1=OP.add,
                )
                out_hi = min(n, s + strip)
                nc.sync.dma_start(
                    out=out_d[r0 : r0 + PB, s:out_hi],
                    in_=ut[:, j["oo"] : j["oo"] + (out_hi - s)],
                )

            # software-pipelined emission: F(k) | M(k-mid_lag) | B(k-back_lag)
            with tc.tile_pool(name="jobs", bufs=bufs) as pool:
                live = []
                for jidx, (r0, s) in enumerate(jobs):
                    live.append(front(pool, r0, s, jidx))
                    if len(live) > mid_lag:
                        mid(live[-1 - mid_lag])
                    if len(live) > back_lag:
                        back(live[-1 - back_lag])
                nj = len(live)
                for k in range(nj - mid_lag, nj):
                    if k >= 0:
                        mid(live[k])
                for k in range(nj - back_lag, nj):
                    if k >= 0:
                        back(live[k])
    return nc


_cached = None


def _get_program():
    global _cached
    if _cached is None:
        nc = bacc.Bacc("TRN2", target_bir_lowering=False, debug=False)
        build_core_program(nc)
        nc.compile()
        _cached = nc
    return _cached


def kernel(alpha: np.ndarray, f: np.ndarray) -> np.ndarray:
    alpha = np.ascontiguousarray(alpha, dtype=np.float32)
    f = np.ascontiguousarray(f, dtype=np.float32).reshape(1, N)
    nc = _get_program()
    fbig = np.ascontiguousarray(np.broadcast_to(f, (PB, N)))
    in_maps = [
        {"alpha": alpha[c * RPC : (c + 1) * RPC], "fbig": fbig}
        for c in range(NCORES)
    ]
    res = bass_utils.run_bass_kernel_spmd(nc, in_maps, core_ids=list(range(NCORES)))
    return np.concatenate([r["out"] for r in res.results], axis=0)


if __name__ == "__main__":
    rng = np.random.default_rng(0)
    a = (0.3 * rng.random((B, N))).astype(np.float32)
    fv = rng.standard_normal(N).astype(np.float32)
    u = kernel(a, fv)
    print(u.shape, u.dtype, np.abs(u).max())


# revision 53
# speedup vs baseline: 1.0185x; 1.0185x over previous
"""Batched tridiagonal (Thomas) solve on 8 TRN2 NeuronCores.

System per row (alpha in [0, 0.3)):
    sub a_i = alpha_{i-1}^2, diag b_i = 1 + alpha_i^3, super c_i = alpha_{i+1}^2 + 2 alpha_{i+1}
Forward elimination denominators denom_i = b_i - g_i/denom_{i-1} (g_i = a_i c_{i-1})
are computed via the linear scan d_i = g_i d_{i-1} + (b_i - 2 g_i), using
1/x ~= 2 - x near 1 (valid: diagonal dominance keeps denom in [0.93, 1.03];
measured end-to-end rel err ~1e-5).  cp/dp/u then come from first-order
recurrences executed with the hardware tensor_tensor_scan instruction.

Sharding: pure data parallel over batch rows (256 rows/core).  Within a core,
rows are split into 128-partition blocks and columns into strips with
contraction halos (forward influence decays ~0.096/step, backward ~0.74/step),
making every (block, strip) job fully independent.
"""

import sys

sys.path.insert(0, "/opt/trn_rl_repo")

import numpy as np

from concourse import bacc, mybir, tile
from concourse import bass_utils
from concourse.ap import AP as bass_AP

F32 = mybir.dt.float32
BF16 = mybir.dt.bfloat16
OP = mybir.AluOpType

B, N = 2048, 8192
NCORES = 8
RPC = B // NCORES          # rows per core
PB = 128                   # partition block (rows per job)
STRIP = 1024               # output columns per job
HALO_L = 8                 # forward-scan warmup (contraction <= 0.0964/step)
HALO_R = 32                # backward-scan warmup (contraction <= 0.739/step)


def _act_reciprocal(nc, out, in_, scale=1.0, bias=0.0):
    """ACT Reciprocal: out = 1/(scale*in). Emitted directly (the bass wrapper
    refuses Reciprocal for generic accuracy reasons; on our inputs, |d| in
    [0.93, 1.03], HW-measured max rel err is 1.2e-5)."""
    se = nc.scalar
    return se.add_instruction(
        mybir.InstActivation(
            name=nc.get_next_instruction_name(),
            func=mybir.ActivationFunctionType.Reciprocal,
            ins=[
                se.lower_ap(in_),
                mybir.ImmediateValue(dtype=mybir.dt.float32, value=bias),
                mybir.ImmediateValue(dtype=mybir.dt.float32, value=scale),
                mybir.ImmediateValue(dtype=mybir.dt.float32, value=0.0),
            ],
            outs=[se.lower_ap(out)],
        )
    )


def build_core_program(nc, rows=RPC, n=N, strip=STRIP, halo_l=HALO_L, halo_r=HALO_R,
                       bufs=6, fr_mode="pool", b_act=True, rnh_act=True,
                       mid_lag=1, back_lag=2, c_alt=0):
    alpha_d = nc.dram_tensor("alpha", [rows, n], F32, kind="ExternalInput").ap()
    fbig_d = nc.dram_tensor("fbig", [PB, n], F32, kind="ExternalInput").ap()
    out_d = nc.dram_tensor("out", [rows, n], F32, kind="ExternalOutput").ap()

    n_blocks = (rows + PB - 1) // PB
    n_strips = (n + strip - 1) // strip
    wmax = halo_l + strip + halo_r

    with tile.TileContext(nc) as tc:
        with tc.tile_pool(name="cpool", bufs=1) as cpool:
            ones = None
            if c_alt:
                ones = cpool.tile([PB, wmax + 2], F32, tag="ones", name="t_ones")
                nc.gpsimd.memset(ones[:], 1.0)
            jobs = []
            for blk in range(n_blocks):
                for si in range(n_strips):
                    jobs.append((blk * PB, si * strip))

            def front(pool, r0, s, jidx=0):
                """DMA + coefficient prep, through g and w."""
                # uniform domain width: edge strips extend their halo inward,
                # so pad columns sit at fixed offsets and slots stay zeroed
                # after their first use.
                w = min(n, wmax)
                dom_lo = max(0, min(s - halo_l, n - w))
                dom_hi = dom_lo + w
                j = {
                    "w": w, "oo": s - dom_lo, "r0": r0, "s": s,
                    "dom_lo": dom_lo, "dom_hi": dom_hi,
                    # padded buffers: col 0 / col w+1 are zero pads for the
                    # shifted reads g_k = A2[k-1]*C[k], ncp_k = -C[k+1]*r_k.
                    "at": pool.tile([PB, wmax + 2], F32, tag="alpha", name="t_alpha"),
                    "a2h": pool.tile([PB, wmax + 2], BF16, tag="a2h", name="t_a2h"),
                    "ch": pool.tile([PB, wmax + 2], BF16, tag="ch", name="t_ch"),
                    "gt": pool.tile([PB, wmax], BF16, tag="g", name="t_g"),
                    "a3t": pool.tile([PB, wmax], F32, tag="a3", name="t_a3"),
                    "bt": pool.tile([PB, wmax + 2], F32, tag="b", name="t_b"),
                    "wt": pool.tile([PB, wmax], F32, tag="w", name="t_w"),
                    "rnh": pool.tile([PB, wmax], BF16, tag="rn", name="t_rn"),
                    "dp": pool.tile([PB, wmax], F32, tag="dp", name="t_dp"),
                    "fbj": pool.tile([PB, wmax], F32, tag="fbj", name="t_fbj"),
                }
                at, a2h, ch = j["at"], j["a2h"], j["ch"]
                nc.gpsimd.memset(at[:, 0:1], 0.0)
                nc.gpsimd.memset(at[:, w + 1 : w + 2], 0.0)
                nc.sync.dma_start(
                    out=at[:, 1 : w + 1], in_=alpha_d[r0 : r0 + PB, dom_lo:dom_hi]
                )
                nc.sync.dma_start(
                    out=j["fbj"][:, 0:w], in_=fbig_d[:, dom_lo:dom_hi]
                )
                # A2 (bf16), S = (alpha+1)^2  (ACT)
                nc.scalar.square(a2h[:, 0 : w + 2], at[:, 0 : w + 2])
                st = j["bt"]  # S staged in b's buffer
                nc.scalar.activation(
                    st[:, 0 : w + 2], at[:, 0 : w + 2],
                    mybir.ActivationFunctionType.Square, bias=1.0, scale=1.0,
                )
                # C = S - 1 = 2 alpha + alpha^2  (bf16; alternate jobs on Pool)
                if c_alt and jidx % 2 == 0:
                    nc.gpsimd.tensor_tensor(
                        out=ch[:, 0 : w + 2], in0=st[:, 0 : w + 2],
                        in1=ones[:, 0 : w + 2], op=OP.subtract,
                    )
                else:
                    nc.vector.tensor_scalar(
                        out=ch[:, 0 : w + 2], in0=st[:, 0 : w + 2], scalar1=-1.0,
                        scalar2=None, op0=OP.add,
                    )
                # A3 = alpha * A2 (Pool, mixed f32 x bf16), b = A3 + 1
                nc.gpsimd.tensor_tensor(
                    out=j["a3t"][:, 0:w], in0=at[:, 1 : w + 1],
                    in1=a2h[:, 1 : w + 1], op=OP.mult,
                )
                # bm2 = b - 2 = A3 - 1  (z-scan right-hand side)
                if b_act:
                    # Copy takes an immediate bias (same path as Reciprocal,
                    # HW-verified); Identity would need a registered const AP.
                    nc.scalar.activation(
                        j["bt"][:, 0:w], j["a3t"][:, 0:w],
                        mybir.ActivationFunctionType.Copy, bias=-1.0, scale=1.0,
                    )
                else:
                    nc.vector.tensor_scalar(
                        out=j["bt"][:, 0:w], in0=j["a3t"][:, 0:w], scalar1=-1.0,
                        scalar2=None, op0=OP.add,
                    )
                # g_k = A2[k-1] * C[k]  (bf16 2x)
                nc.vector.tensor_tensor(
                    out=j["gt"][:, 0:w], in0=a2h[:, 0:w], in1=ch[:, 1 : w + 1],
                    op=OP.mult,
                )
                return j

            def mid(j):
                """z-scan (z = d - 2), then rn = 1/(-z-2) = -1/d on ACT."""
                w = j["w"]
                zt = j["a3t"]  # A3 dead after bm2
                nc.vector.tensor_tensor_scan(
                    out=zt[:, 0:w], data0=j["gt"][:, 0:w], data1=j["bt"][:, 0:w],
                    initial=0.0, op0=OP.mult, op1=OP.add,
                )
                rn = j["bt"]  # bm2 dead after z-scan; rn = -1/d (f32)
                _act_reciprocal(nc, rn[:, 0:w], zt[:, 0:w], scale=-1.0, bias=-2.0)
                if rnh_act:
                    nc.scalar.mul(j["rnh"][:, 0:w], rn[:, 0:w], 1.0)
                else:
                    nc.vector.tensor_scalar(
                        out=j["rnh"][:, 0:w], in0=rn[:, 0:w], scalar1=1.0,
                        scalar2=None, op0=OP.mult,
                    )

            def back(j):
                """ar', fr', dp-scan, ncp, u-scan, output DMA."""
                w, r0, s = j["w"], j["r0"], j["s"]
                at, a2h, ch, gt, rn = j["at"], j["a2h"], j["ch"], j["gt"], j["bt"]
                # ar'_k = A2[k-1] * rn_k  (bf16 2x, into gt; g dead)
                nc.vector.tensor_tensor(
                    out=gt[:, 0:w], in0=a2h[:, 0:w], in1=j["rnh"][:, 0:w],
                    op=OP.mult,
                )
                # fr'_k = f_k * rn_k = -f_k r_k  (into wt; w dead)
                fr = j["wt"]
                eng = nc.gpsimd if fr_mode == "pool" else nc.vector
                eng.tensor_tensor(
                    out=fr[:, 0:w], in0=j["fbj"][:, 0:w],
                    in1=rn[:, 0:w], op=OP.mult,
                )
                # dp-scan: dp_k = ar'_k * dp_{k-1} - fr'_k  (dp positive)
                nc.vector.tensor_tensor_scan(
                    out=j["dp"][:, 0:w], data0=gt[:, 0:w], data1=fr[:, 0:w],
                    initial=0.0, op0=OP.mult, op1=OP.subtract,
                )
                # ncp_k = C[k+1] * rn_k  (bf16 2x, into a2h; dead after ar')
                ncp = a2h
                nc.vector.tensor_tensor(
                    out=ncp[:, 0:w], in0=ch[:, 2 : w + 2], in1=j["rnh"][:, 0:w],
                    op=OP.mult,
                )
                # u-scan (backward): u_k = ncp_k * u_{k+1} + dp_k  (into wt)
                ut = j["wt"]
                nc.vector.tensor_tensor_scan(
                    out=ut[:, 0:w][:, ::-1],
                    data0=ncp[:, 0:w][:, ::-1],
                    data1=j["dp"][:, 0:w][:, ::-1],
                    initial=0.0, op0=OP.mult, op1=OP.add,
                )
                out_hi = min(n, s + strip)
                nc.sync.dma_start(
                    out=out_d[r0 : r0 + PB, s:out_hi],
                    in_=ut[:, j["oo"] : j["oo"] + (out_hi - s)],
                )

            # software-pipelined emission: F(k) | M(k-mid_lag) | B(k-back_lag)
            with tc.tile_pool(name="jobs", bufs=bufs) as pool:
                live = []
                for jidx, (r0, s) in enumerate(jobs):
                    live.append(front(pool, r0, s, jidx))
                    if len(live) > mid_lag:
                        mid(live[-1 - mid_lag])
                    if len(live) > back_lag:
                        back(live[-1 - back_lag])
                nj = len(live)
                for k in range(nj - mid_lag, nj):
                    if k >= 0:
                        mid(live[k])
                for k in range(nj - back_lag, nj):
                    if k >= 0:
                        back(live[k])
    return nc


_cached = None


def _get_program():
    global _cached
    if _cached is None:
        nc = bacc.Bacc("TRN2", target_bir_lowering=False, debug=False)
        build_core_program(nc)
        nc.compile()
        _cached = nc
    return _cached


def kernel(alpha: np.ndarray, f: np.ndarray) -> np.ndarray:
    alpha = np.ascontiguousarray(alpha, dtype=np.float32)
    f = np.ascontiguousarray(f, dtype=np.float32).reshape(1, N)
    nc = _get_program()
    fbig = np.ascontiguousarray(np.broadcast_to(f, (PB, N)))
    in_maps = [
        {"alpha": alpha[c * RPC : (c + 1) * RPC], "fbig": fbig}
        for c in range(NCORES)
    ]
    res = bass_utils.run_bass_kernel_spmd(nc, in_maps, core_ids=list(range(NCORES)))
    return np.concatenate([r["out"] for r in res.results], axis=0)


if __name__ == "__main__":
    rng = np.random.default_rng(0)
    a = (0.3 * rng.random((B, N))).astype(np.float32)
    fv = rng.standard_normal(N).astype(np.float32)
    u = kernel(a, fv)
    print(u.shape, u.dtype, np.abs(u).max())


# revision 54
# speedup vs baseline: 1.0225x; 1.0039x over previous
"""Batched tridiagonal (Thomas) solve on 8 TRN2 NeuronCores.

System per row (alpha in [0, 0.3)):
    sub a_i = alpha_{i-1}^2, diag b_i = 1 + alpha_i^3, super c_i = alpha_{i+1}^2 + 2 alpha_{i+1}
Forward elimination denominators denom_i = b_i - g_i/denom_{i-1} (g_i = a_i c_{i-1})
are computed via the linear scan d_i = g_i d_{i-1} + (b_i - 2 g_i), using
1/x ~= 2 - x near 1 (valid: diagonal dominance keeps denom in [0.93, 1.03];
measured end-to-end rel err ~1e-5).  cp/dp/u then come from first-order
recurrences executed with the hardware tensor_tensor_scan instruction.

Sharding: pure data parallel over batch rows (256 rows/core).  Within a core,
rows are split into 128-partition blocks and columns into strips with
contraction halos (forward influence decays ~0.096/step, backward ~0.74/step),
making every (block, strip) job fully independent.
"""

import sys

sys.path.insert(0, "/opt/trn_rl_repo")

import numpy as np

from concourse import bacc, mybir, tile
from concourse import bass_utils
from concourse.ap import AP as bass_AP

F32 = mybir.dt.float32
BF16 = mybir.dt.bfloat16
OP = mybir.AluOpType

B, N = 2048, 8192
NCORES = 8
RPC = B // NCORES          # rows per core
PB = 128                   # partition block (rows per job)
STRIP = 1024               # output columns per job
HALO_L = 8                 # forward-scan warmup (contraction <= 0.0964/step)
HALO_R = 32                # backward-scan warmup (contraction <= 0.739/step)


def _act_reciprocal(nc, out, in_, scale=1.0, bias=0.0):
    """ACT Reciprocal: out = 1/(scale*in). Emitted directly (the bass wrapper
    refuses Reciprocal for generic accuracy reasons; on our inputs, |d| in
    [0.93, 1.03], HW-measured max rel err is 1.2e-5)."""
    se = nc.scalar
    return se.add_instruction(
        mybir.InstActivation(
            name=nc.get_next_instruction_name(),
            func=mybir.ActivationFunctionType.Reciprocal,
            ins=[
                se.lower_ap(in_),
                mybir.ImmediateValue(dtype=mybir.dt.float32, value=bias),
                mybir.ImmediateValue(dtype=mybir.dt.float32, value=scale),
                mybir.ImmediateValue(dtype=mybir.dt.float32, value=0.0),
            ],
            outs=[se.lower_ap(out)],
        )
    )


def build_core_program(nc, rows=RPC, n=N, strip=STRIP, halo_l=HALO_L, halo_r=HALO_R,
                       bufs=6, fr_mode="pool", b_act=True, rnh_act=True,
                       mid_lag=1, back_lag=2, c_alt=0):
    alpha_d = nc.dram_tensor("alpha", [rows, n], F32, kind="ExternalInput").ap()
    fbig_d = nc.dram_tensor("fbig", [PB, n], F32, kind="ExternalInput").ap()
    out_d = nc.dram_tensor("out", [rows, n], F32, kind="ExternalOutput").ap()

    n_blocks = (rows + PB - 1) // PB
    n_strips = (n + strip - 1) // strip
    wmax = halo_l + strip + halo_r

    with tile.TileContext(nc) as tc:
        with tc.tile_pool(name="cpool", bufs=1) as cpool:
            ones = None
            if c_alt:
                ones = cpool.tile([PB, wmax + 2], F32, tag="ones", name="t_ones")
                nc.gpsimd.memset(ones[:], 1.0)
            jobs = []
            for blk in range(n_blocks):
                for si in range(n_strips):
                    jobs.append((blk * PB, si * strip))

            def front(pool, r0, s, jidx=0):
                """DMA + coefficient prep, through g and w."""
                # uniform domain width: edge strips extend their halo inward,
                # so pad columns sit at fixed offsets and slots stay zeroed
                # after their first use.
                w = min(n, wmax)
                dom_lo = max(0, min(s - halo_l, n - w))
                dom_hi = dom_lo + w
                j = {
                    "w": w, "oo": s - dom_lo, "r0": r0, "s": s,
                    "dom_lo": dom_lo, "dom_hi": dom_hi,
                    # padded buffers: col 0 / col w+1 are zero pads for the
                    # shifted reads g_k = A2[k-1]*C[k], ncp_k = -C[k+1]*r_k.
                    "at": pool.tile([PB, wmax + 2], F32, tag="alpha", name="t_alpha"),
                    "a2h": pool.tile([PB, wmax + 2], BF16, tag="a2h", name="t_a2h"),
                    "ch": pool.tile([PB, wmax + 2], BF16, tag="ch", name="t_ch"),
                    "gt": pool.tile([PB, wmax], BF16, tag="g", name="t_g"),
                    "a3t": pool.tile([PB, wmax], F32, tag="a3", name="t_a3"),
                    "bt": pool.tile([PB, wmax + 2], F32, tag="b", name="t_b"),
                    "wt": pool.tile([PB, wmax], F32, tag="w", name="t_w"),
                    "rnh": pool.tile([PB, wmax], BF16, tag="rn", name="t_rn"),
                    "dp": pool.tile([PB, wmax], F32, tag="dp", name="t_dp"),
                    "fbj": pool.tile([PB, wmax], F32, tag="fbj", name="t_fbj"),
                }
                at, a2h, ch = j["at"], j["a2h"], j["ch"]
                nc.gpsimd.memset(at[:, 0:1], 0.0)
                nc.gpsimd.memset(at[:, w + 1 : w + 2], 0.0)
                nc.sync.dma_start(
                    out=at[:, 1 : w + 1], in_=alpha_d[r0 : r0 + PB, dom_lo:dom_hi]
                )
                nc.sync.dma_start(
                    out=j["fbj"][:, 0:w], in_=fbig_d[:, dom_lo:dom_hi]
                )
                # A2 (bf16), S = (alpha+1)^2  (ACT)
                nc.scalar.square(a2h[:, 0 : w + 2], at[:, 0 : w + 2])
                st = j["bt"]  # S staged in b's buffer
                nc.scalar.activation(
                    st[:, 0 : w + 2], at[:, 0 : w + 2],
                    mybir.ActivationFunctionType.Square, bias=1.0, scale=1.0,
                )
                # C = S - 1 = 2 alpha + alpha^2  (bf16; alternate jobs on Pool)
                if c_alt and jidx % 2 == 0:
                    nc.gpsimd.tensor_tensor(
                        out=ch[:, 0 : w + 2], in0=st[:, 0 : w + 2],
                        in1=ones[:, 0 : w + 2], op=OP.subtract,
                    )
                else:
                    nc.vector.tensor_scalar(
                        out=ch[:, 0 : w + 2], in0=st[:, 0 : w + 2], scalar1=-1.0,
                        scalar2=None, op0=OP.add,
                    )
                # A3 = alpha * A2 (Pool, mixed f32 x bf16), b = A3 + 1
                nc.gpsimd.tensor_tensor(
                    out=j["a3t"][:, 0:w], in0=at[:, 1 : w + 1],
                    in1=a2h[:, 1 : w + 1], op=OP.mult,
                )
                # g_k = A2[k-1] * C[k]  (bf16 2x)
                nc.vector.tensor_tensor(
                    out=j["gt"][:, 0:w], in0=a2h[:, 0:w], in1=ch[:, 1 : w + 1],
                    op=OP.mult,
                )
                return j

            def mid(j):
                """bm2, z-scan (z = d - 2), then rn = 1/(-z-2) = -1/d on ACT."""
                w = j["w"]
                # bm2 = b - 2 = A3 - 1: emitted here (not in front) so it does
                # not head-of-line block ACT behind the Pool A3 dependency.
                if b_act:
                    nc.scalar.activation(
                        j["bt"][:, 0:w], j["a3t"][:, 0:w],
                        mybir.ActivationFunctionType.Copy, bias=-1.0, scale=1.0,
                    )
                else:
                    nc.vector.tensor_scalar(
                        out=j["bt"][:, 0:w], in0=j["a3t"][:, 0:w], scalar1=-1.0,
                        scalar2=None, op0=OP.add,
                    )
                zt = j["a3t"]  # A3 dead after bm2
                nc.vector.tensor_tensor_scan(
                    out=zt[:, 0:w], data0=j["gt"][:, 0:w], data1=j["bt"][:, 0:w],
                    initial=0.0, op0=OP.mult, op1=OP.add,
                )
                rn = j["bt"]  # bm2 dead after z-scan; rn = -1/d (f32)
                _act_reciprocal(nc, rn[:, 0:w], zt[:, 0:w], scale=-1.0, bias=-2.0)
                if rnh_act:
                    nc.scalar.mul(j["rnh"][:, 0:w], rn[:, 0:w], 1.0)
                else:
                    nc.vector.tensor_scalar(
                        out=j["rnh"][:, 0:w], in0=rn[:, 0:w], scalar1=1.0,
                        scalar2=None, op0=OP.mult,
                    )

            def back(j):
                """ar', fr', dp-scan, ncp, u-scan, output DMA."""
                w, r0, s = j["w"], j["r0"], j["s"]
                at, a2h, ch, gt, rn = j["at"], j["a2h"], j["ch"], j["gt"], j["bt"]
                # ar'_k = A2[k-1] * rn_k  (bf16 2x, into gt; g dead)
                nc.vector.tensor_tensor(
                    out=gt[:, 0:w], in0=a2h[:, 0:w], in1=j["rnh"][:, 0:w],
                    op=OP.mult,
                )
                # fr'_k = f_k * rn_k = -f_k r_k  (into wt; w dead)
                fr = j["wt"]
                eng = nc.gpsimd if fr_mode == "pool" else nc.vector
                eng.tensor_tensor(
                    out=fr[:, 0:w], in0=j["fbj"][:, 0:w],
                    in1=rn[:, 0:w], op=OP.mult,
                )
                # dp-scan: dp_k = ar'_k * dp_{k-1} - fr'_k  (dp positive)
                nc.vector.tensor_tensor_scan(
                    out=j["dp"][:, 0:w], data0=gt[:, 0:w], data1=fr[:, 0:w],
                    initial=0.0, op0=OP.mult, op1=OP.subtract,
                )
                # ncp_k = C[k+1] * rn_k  (bf16 2x, into a2h; dead after ar')
                ncp = a2h
                nc.vector.tensor_tensor(
                    out=ncp[:, 0:w], in0=ch[:, 2 : w + 2], in1=j["rnh"][:, 0:w],
                    op=OP.mult,
                )
                # u-scan (backward): u_k = ncp_k * u_{k+1} + dp_k  (into wt)
                ut = j["wt"]
                nc.vector.tensor_tensor_scan(
                    out=ut[:, 0:w][:, ::-1],
                    data0=ncp[:, 0:w][:, ::-1],
                    data1=j["dp"][:, 0:w][:, ::-1],
                    initial=0.0, op0=OP.mult, op1=OP.add,
                )
                out_hi = min(n, s + strip)
                nc.sync.dma_start(
                    out=out_d[r0 : r0 + PB, s:out_hi],
                    in_=ut[:, j["oo"] : j["oo"] + (out_hi - s)],
                )

            # software-pipelined emission: F(k) | M(k-mid_lag) | B(k-back_lag)
            with tc.tile_pool(name="jobs", bufs=bufs) as pool:
                live = []
                for jidx, (r0, s) in enumerate(jobs):
                    live.append(front(pool, r0, s, jidx))
                    if len(live) > mid_lag:
                        mid(live[-1 - mid_lag])
                    if len(live) > back_lag:
                        back(live[-1 - back_lag])
                nj = len(live)
                for k in range(nj - mid_lag, nj):
                    if k >= 0:
                        mid(live[k])
                for k in range(nj - back_lag, nj):
                    if k >= 0:
                        back(live[k])
    return nc


_cached = None


def _get_program():
    global _cached
    if _cached is None:
        nc = bacc.Bacc("TRN2", target_bir_lowering=False, debug=False)
        build_core_program(nc)
        nc.compile()
        _cached = nc
    return _cached


def kernel(alpha: np.ndarray, f: np.ndarray) -> np.ndarray:
    alpha = np.ascontiguousarray(alpha, dtype=np.float32)
    f = np.ascontiguousarray(f, dtype=np.float32).reshape(1, N)
    nc = _get_program()
    fbig = np.ascontiguousarray(np.broadcast_to(f, (PB, N)))
    in_maps = [
        {"alpha": alpha[c * RPC : (c + 1) * RPC], "fbig": fbig}
        for c in range(NCORES)
    ]
    res = bass_utils.run_bass_kernel_spmd(nc, in_maps, core_ids=list(range(NCORES)))
    return np.concatenate([r["out"] for r in res.results], axis=0)


if __name__ == "__main__":
    rng = np.random.default_rng(0)
    a = (0.3 * rng.random((B, N))).astype(np.float32)
    fv = rng.standard_normal(N).astype(np.float32)
    u = kernel(a, fv)
    print(u.shape, u.dtype, np.abs(u).max())
